# revision 17
# baseline (speedup 1.0000x reference)
"""CRQVAE (encoder MLP + 4-level residual VQ) on 8 TRN2 NeuronCores.

Data-parallel: batch N=131072 sharded 8 ways (16384 rows/core); encoder
weights + codebooks replicated. The scalar rq_loss is reduced on the host
from tiny per-core partials (the only cross-core communication).

Device design (per core, feature-major activations, 32 tiles x 512 cols):
  - All large matmuls run as 3-term float32r splits (a = ah + al with
    ah = round-to-11-mantissa-bits): ah.bh + ah.bl + al.bh at 1 cyc/row
    gives ~fp32 accuracy at 1/4 the fp32 matmul cost. Weight/x splits are
    precomputed on the host; hidden-activation splits on device
    (ACT rounding copy + GpSimd subtract).
  - RVQ per level: batch-major scores s = 2 r.c - |c|^2 (3-term f32r,
    residual stationary), DVE row-max m, one-hot = is_ge(s, m) on the SAME
    tensor (exact argmax one-hot by construction), PE-transpose of the
    bf16 one-hot to code-major, q/idx gather as a 3-term bf16 split of
    [cb | 0 | iota] (exact, since one-hot is {0,1}), residual update on DVE.
  - Outputs: x_qT, zT (feature-major), idx rows (f32), m row-maxima (host
    loss telescope: sum|r_{l+1}|^2 = sum|z|^2 - sum_{j<=l} sum_n m_j).
"""
import os
import sys

sys.path.insert(0, "/opt/trn_rl_repo")

import numpy as np

import concourse.bass as bass
import concourse.tile as tile
from concourse import bacc, mybir
from concourse import bass_utils

F32 = mybir.dt.float32
F32R = mybir.dt.float32r
BF16 = mybir.dt.bfloat16
AF = mybir.ActivationFunctionType

N_CORES = 8
N_TOTAL = 131072
N_PER_CORE = N_TOTAL // N_CORES        # 16384
B = 512                                # batch columns per tile
N_TILES = N_PER_CORE // B              # 32
D0, D1, D2, E = 768, 512, 256, 64
K = 256                                # codebook size
L = 4                                  # rvq levels
BETA = 0.25

KC1, KC2, KC3 = D0 // 128, D1 // 128, D2 // 128   # 6, 4, 2
JC1, JC2 = D1 // 128, D2 // 128                   # 4, 2
NC = B // 128                                     # 4 n-chunks per tile
KCC = K // 128                                    # 2 code chunks


def build_kernel(n_tiles=N_TILES):
    nc = bacc.Bacc("TRN2", target_bir_lowering=False, debug=False)
    n = n_tiles * B

    # ---------------- DRAM parameters ----------------
    xh_d = nc.dram_tensor("xh", [D0, n], F32R, kind="ExternalInput")
    xl_d = nc.dram_tensor("xl", [D0, n], F32R, kind="ExternalInput")
    w0h_d = nc.dram_tensor("w0h", [D0, D1], F32R, kind="ExternalInput")
    w0l_d = nc.dram_tensor("w0l", [D0, D1], F32R, kind="ExternalInput")
    w1h_d = nc.dram_tensor("w1h", [D1, D2], F32R, kind="ExternalInput")
    w1l_d = nc.dram_tensor("w1l", [D1, D2], F32R, kind="ExternalInput")
    w2h_d = nc.dram_tensor("w2h", [D2, E + 1], F32R, kind="ExternalInput")
    w2l_d = nc.dram_tensor("w2l", [D2, E + 1], F32R, kind="ExternalInput")
    b0_d = nc.dram_tensor("b0", [128, JC1], F32, kind="ExternalInput")
    b1_d = nc.dram_tensor("b1", [128, JC2], F32, kind="ExternalInput")
    b2_d = nc.dram_tensor("b2", [E + 1, 1], F32, kind="ExternalInput")   # [b2;1]
    cbah_d = nc.dram_tensor("cbah", [L, E + 1, K], F32R, kind="ExternalInput")
    cbal_d = nc.dram_tensor("cbal", [L, E + 1, K], F32R, kind="ExternalInput")
    cbx_d = nc.dram_tensor("cbx", [3, L, K, 97], BF16, kind="ExternalInput")
    eyeb_d = nc.dram_tensor("eyeb", [128, 128], BF16, kind="ExternalInput")

    xqT_d = nc.dram_tensor("xqT", [E, n], F32, kind="ExternalOutput")
    zT_d = nc.dram_tensor("zT", [E, n], F32, kind="ExternalOutput")
    idx_d = nc.dram_tensor("idx", [L, n], BF16, kind="ExternalOutput")
    mv_d = nc.dram_tensor("mv", [128, n_tiles * L * NC], F32, kind="ExternalOutput")

    with tile.TileContext(nc) as tc:
        with (
            tc.tile_pool(name="const", bufs=1) as cp,
            tc.tile_pool(name="xin", bufs=2) as xp,
            tc.tile_pool(name="act", bufs=2) as ap,
            tc.tile_pool(name="rres", bufs=6) as rp,
            tc.tile_pool(name="oh", bufs=2) as ohp,
            tc.tile_pool(name="stage", bufs=2) as stp,
            tc.tile_pool(name="pmlp", bufs=2, space="PSUM") as pmlp,
            tc.tile_pool(name="pbs", bufs=2, space="PSUM") as pbs,
            tc.tile_pool(name="pfs", bufs=1, space="PSUM") as pfs,
            tc.tile_pool(name="pq", bufs=1, space="PSUM") as pq,
        ):
            # ---- constants (loaded once) ----
            w0h = cp.tile([128, KC1, D1], F32R, tag="w0h")
            w0l = cp.tile([128, KC1, D1], F32R, tag="w0l")
            w1h = cp.tile([128, KC2, D2], F32R, tag="w1h")
            w1l = cp.tile([128, KC2, D2], F32R, tag="w1l")
            w2h = cp.tile([128, KC3, E + 1], F32R, tag="w2h")
            w2l = cp.tile([128, KC3, E + 1], F32R, tag="w2l")
            b0c = cp.tile([128, JC1], F32, tag="b0")
            b1c = cp.tile([128, JC2], F32, tag="b1")
            b2c = cp.tile([E + 1, 1], F32, tag="b2")
            eyeb = cp.tile([128, 128], BF16, tag="eyeb")
            cbah = [cp.tile([E + 1, K], F32R, tag=f"cbah{l}", name=f"cbah{l}")
                    for l in range(L)]
            cbal = [cp.tile([E + 1, K], F32R, tag=f"cbal{l}", name=f"cbal{l}")
                    for l in range(L)]
            cbx = [cp.tile([128, KCC, 3, 97], BF16, tag=f"cbx{l}", name=f"cbx{l}")
                   for l in range(L)]

            for c in range(KC1):
                nc.sync.dma_start(w0h[:, c, :], w0h_d[c * 128:(c + 1) * 128, :])
                nc.sync.dma_start(w0l[:, c, :], w0l_d[c * 128:(c + 1) * 128, :])
            for c in range(KC2):
                nc.sync.dma_start(w1h[:, c, :], w1h_d[c * 128:(c + 1) * 128, :])
                nc.sync.dma_start(w1l[:, c, :], w1l_d[c * 128:(c + 1) * 128, :])
            for c in range(KC3):
                nc.sync.dma_start(w2h[:, c, :], w2h_d[c * 128:(c + 1) * 128, :])
                nc.sync.dma_start(w2l[:, c, :], w2l_d[c * 128:(c + 1) * 128, :])
            nc.sync.dma_start(b0c[:], b0_d[:])
            nc.sync.dma_start(b1c[:], b1_d[:])
            nc.sync.dma_start(b2c[:], b2_d[:])
            nc.sync.dma_start(eyeb[:], eyeb_d[:])
            for l in range(L):
                nc.sync.dma_start(cbah[l][:], cbah_d[l])
                nc.sync.dma_start(cbal[l][:], cbal_d[l])
                for t in range(3):
                    for kc in range(KCC):
                        nc.sync.dma_start(cbx[l][:, kc, t, :],
                                          cbx_d[t, l, kc * 128:(kc + 1) * 128, :])

            for i in range(n_tiles):
                sl = slice(i * B, (i + 1) * B)

                # ---- load xT tiles (hi/lo) ----
                xth = xp.tile([128, KC1, B], F32R, tag="xh")
                xtl = xp.tile([128, KC1, B], F32R, tag="xl")
                for c in range(KC1):
                    nc.sync.dma_start(xth[:, c, :], xh_d[c * 128:(c + 1) * 128, sl])
                    nc.sync.dma_start(xtl[:, c, :], xl_d[c * 128:(c + 1) * 128, sl])

                def mm3(pm, wh, wl, rh, rl, nk):
                    """3-term f32r accumulation over nk contraction chunks."""
                    first = True
                    for c in range(nk):
                        for (wt, rt) in ((wh, rh), (wl, rh), (wh, rl)):
                            last = (c == nk - 1) and (wt is wh and rt is rl)
                            nc.tensor.matmul(pm, wt[0][:, c, wt[1]], rt[:, c, :],
                                             start=first, stop=last)
                            first = False

                # ---- L1: h1 = relu(W0.T @ x + b0) ----
                h1 = ap.tile([128, JC1, B], F32, tag="h1")
                for j in range(JC1):
                    pm = pmlp.tile([128, B], F32, tag="pm")
                    jsl = slice(j * 128, (j + 1) * 128)
                    mm3(pm[:], (w0h, jsl), (w0l, jsl), xth, xtl, KC1)
                    nc.scalar.activation(h1[:, j, :], pm[:], AF.Relu,
                                         bias=b0c[:, j:j + 1], scale=1.0)
                h1h = ap.tile([128, JC1, B], F32R, tag="h1h")
                nc.scalar.copy(h1h[:], h1[:])
                h1l = ap.tile([128, JC1, B], F32R, tag="h1l")
                nc.gpsimd.tensor_tensor(h1l[:], h1[:], h1h[:],
                                        op=mybir.AluOpType.subtract)

                # ---- L2 ----
                h2 = ap.tile([128, JC2, B], F32, tag="h2", bufs=1)
                for j in range(JC2):
                    pm = pmlp.tile([128, B], F32, tag="pm")
                    jsl = slice(j * 128, (j + 1) * 128)
                    mm3(pm[:], (w1h, jsl), (w1l, jsl), h1h, h1l, KC2)
                    nc.scalar.activation(h2[:, j, :], pm[:], AF.Relu,
                                         bias=b1c[:, j:j + 1], scale=1.0)
                h2h = ap.tile([128, JC2, B], F32R, tag="h2h", bufs=1)
                nc.scalar.copy(h2h[:], h2[:])
                h2l = ap.tile([128, JC2, B], F32R, tag="h2l", bufs=1)
                nc.gpsimd.tensor_tensor(h2l[:], h2[:], h2h[:],
                                        op=mybir.AluOpType.subtract)

                # ---- L3: z'' = [z + b2; 1] ----
                pzt = pmlp.tile([E + 1, B], F32, tag="pm")
                asl = slice(0, E + 1)
                mm3(pzt[:], (w2h, asl), (w2l, asl), h2h, h2l, KC3)
                r0 = rp.tile([E + 1, B], F32, tag="r")
                nc.scalar.activation(r0[:], pzt[:], AF.Identity,
                                     bias=b2c[:], scale=1.0)
                nc.sync.dma_start(zT_d[:, sl], r0[0:E, :])

                mst = stp.tile([128, L * NC], F32, tag="mst")

                r_cur = r0
                for l in range(L):
                    # split r into f32r hi/lo
                    rh = rp.tile([E + 1, B], F32R, tag="rh")
                    nc.scalar.copy(rh[:], r_cur[:])
                    rl = rp.tile([E + 1, B], F32R, tag="rl")
                    nc.gpsimd.tensor_tensor(rl[:], r_cur[:], rh[:],
                                            op=mybir.AluOpType.subtract)

                    # ---- batch-major scores s = 2 r.c - |c|^2 -> row max ----
                    pb = pbs.tile([128, NC, K], F32, tag="pb")
                    for c in range(NC):
                        csl = slice(c * 128, (c + 1) * 128)
                        nc.tensor.matmul(pb[:, c, :], rh[:, csl], cbah[l][:],
                                         start=True, stop=False)
                        nc.tensor.matmul(pb[:, c, :], rh[:, csl], cbal[l][:],
                                         start=False, stop=False)
                        nc.tensor.matmul(pb[:, c, :], rl[:, csl], cbah[l][:],
                                         start=False, stop=True)
                    nc.vector.tensor_reduce(mst[:, l * NC:(l + 1) * NC], pb[:],
                                            axis=mybir.AxisListType.X,
                                            op=mybir.AluOpType.max)

                    # ---- exact one-hot: (s >= rowmax), bf16 ----
                    oh = ohp.tile([128, NC, K], BF16, tag="oh")
                    for c in range(NC):
                        nc.vector.tensor_scalar(
                            oh[:, c, :], pb[:, c, :],
                            scalar1=mst[:, l * NC + c:l * NC + c + 1],
                            scalar2=None, op0=mybir.AluOpType.is_ge)

                    # ---- transpose one-hot to code-major (PE, bf16) ----
                    pt = pfs.tile([128, KCC, B], BF16, tag="pt")
                    for c in range(NC):
                        for kc in range(KCC):
                            nc.tensor.transpose(
                                pt[:, kc, c * 128:(c + 1) * 128],
                                oh[:, c, kc * 128:(kc + 1) * 128], eyeb[:])
                    ohT = ohp.tile([128, KCC, B], BF16, tag="ohT")
                    nc.scalar.copy(ohT[:], pt[:])

                    # ---- q / idx gather: 3-term bf16 split of cb (exact) ----
                    pqt = pq.tile([97, B], F32, tag="pq")
                    for t in range(3):
                        for kc in range(KCC):
                            nc.tensor.matmul(pqt[:], cbx[l][:, kc, t, :],
                                             ohT[:, kc, :],
                                             start=(t == 0 and kc == 0),
                                             stop=(t == 2 and kc == KCC - 1))
                    ixl = stp.tile([1, B], BF16, tag=f"ist{l}", name=f"ist{l}", bufs=1)
                    nc.scalar.copy(ixl[:], pqt[96:97, :])
                    nc.gpsimd.dma_start(idx_d[l:l + 1, sl], ixl[:])

                    # ---- residual update (rows 0..64; row64: 1-0=1) ----
                    r_nxt = rp.tile([E + 1, B], F32, tag="r")
                    nc.vector.tensor_tensor(r_nxt[:], r_cur[:], pqt[0:E + 1, :],
                                            op=mybir.AluOpType.subtract)
                    r_cur = r_nxt

                # ---- x_q = z - r4 ----
                xq = stp.tile([E, B], F32, tag="xq", bufs=1)
                nc.vector.tensor_tensor(xq[:], r0[0:E, :], r_cur[0:E, :],
                                        op=mybir.AluOpType.subtract)
                nc.gpsimd.dma_start(xqT_d[:, sl], xq[:])
                nc.gpsimd.dma_start(mv_d[:, i * L * NC:(i + 1) * L * NC], mst[:])

    nc.compile()
    return nc


def _r11(x):
    """Round fp32 to 11 explicit mantissa bits (matches HW f32r width)."""
    i = x.view(np.int32).astype(np.int64)
    i = (i + (1 << 11)) & ~((1 << 12) - 1)
    return np.asarray(i, np.int64).astype(np.int32).view(np.float32)


def _split(x):
    h = _r11(np.ascontiguousarray(x, np.float32))
    l = (x - h).astype(np.float32)
    return h, np.ascontiguousarray(l)


def _host_prep(x, W0, b0, W1, b1, W2, b2, codebooks):
    """Build per-core input maps."""
    import ml_dtypes
    cb = np.asarray(codebooks, np.float32)                    # [4,256,64]
    cn32 = ((cb ** 2).sum(-1)).astype(np.float32)             # fp32 |c|^2

    cba = np.zeros((L, E + 1, K), np.float32)
    cba[:, :E, :] = np.transpose(cb, (0, 2, 1)) * 2.0
    cba[:, E, :] = -cn32
    cbah, cbal = _split(cba)

    cbf = np.zeros((L, K, 97), np.float32)
    cbf[:, :, :E] = cb
    cbf[:, :, 96] = np.arange(K, dtype=np.float32)[None, :]
    h = cbf.astype(ml_dtypes.bfloat16)
    r1 = cbf - h.astype(np.float32)
    m_ = r1.astype(ml_dtypes.bfloat16)
    r2 = r1 - m_.astype(np.float32)
    lo = r2.astype(ml_dtypes.bfloat16)
    cbx = np.stack([h, m_, lo])                               # [3,L,K,97] bf16
    eyeb = np.eye(128, dtype=ml_dtypes.bfloat16)

    w2a = np.zeros((D2, E + 1), np.float32)
    w2a[:, :E] = np.asarray(W2, np.float32)
    b2a = np.zeros((E + 1, 1), np.float32)
    b2a[:E, 0] = np.asarray(b2, np.float32)
    b2a[E, 0] = 1.0

    w0h, w0l = _split(np.asarray(W0, np.float32))
    w1h, w1l = _split(np.asarray(W1, np.float32))
    w2h, w2l = _split(w2a)

    common = {
        "w0h": w0h, "w0l": w0l, "w1h": w1h, "w1l": w1l,
        "w2h": w2h, "w2l": w2l,
        "b0": np.ascontiguousarray(np.asarray(b0, np.float32).reshape(JC1, 128).T),
        "b1": np.ascontiguousarray(np.asarray(b1, np.float32).reshape(JC2, 128).T),
        "b2": b2a,
        "cbah": cbah, "cbal": cbal, "cbx": cbx, "eyeb": eyeb,
    }

    x = np.asarray(x, np.float32)
    in_maps = []
    for c in range(N_CORES):
        xs = x[c * N_PER_CORE:(c + 1) * N_PER_CORE]
        xT = np.ascontiguousarray(xs.T)
        xh, xl = _split(xT)
        m = dict(common)
        m["xh"] = xh
        m["xl"] = xl
        in_maps.append(m)
    return in_maps


_NC_CACHE = {}


class _Res:
    def __init__(self, results):
        self.results = results


def _run_pjrt(nc, in_maps, n_cores, time_reps=0):
    """Execute the compiled Bass graph on n_cores via PJRT (axon), optionally
    timing warm repeat executions with device-resident inputs."""
    import time as _time
    import jax
    from jax.sharding import Mesh, PartitionSpec
    from jax.experimental.shard_map import shard_map
    from concourse import bass2jax, mybir as _mb
    from concourse.bass2jax import _bass_exec_p, install_neuronx_cc_hook

    install_neuronx_cc_hook()
    partition_name = nc.partition_id_tensor.name if nc.partition_id_tensor else None

    in_names, out_names, out_avals, zero_outs = [], [], [], []
    for alloc in nc.m.functions[0].allocations:
        if not isinstance(alloc, _mb.MemoryLocationSet):
            continue
        name = alloc.memorylocations[0].name
        if alloc.kind == "ExternalInput":
            if name != partition_name:
                in_names.append(name)
        elif alloc.kind == "ExternalOutput":
            shape = tuple(alloc.tensor_shape)
            dtype = _mb.dt.np(alloc.dtype)
            out_names.append(name)
            out_avals.append(jax.core.ShapedArray(shape, dtype))
            zero_outs.append(np.zeros(shape, dtype))
    n_params = len(in_names)
    n_outs = len(out_avals)
    all_in_names = list(in_names) + list(out_names)
    if partition_name is not None:
        all_in_names.append(partition_name)

    def _body(*args):
        operands = list(args)
        if partition_name is not None:
            operands.append(bass2jax.partition_id_tensor())
        outs = _bass_exec_p.bind(
            *operands,
            out_avals=tuple(out_avals),
            in_names=tuple(all_in_names),
            out_names=tuple(out_names),
            lowering_input_output_aliases=(),
            sim_require_finite=True,
            sim_require_nnan=True,
            nc=nc,
        )
        return tuple(outs)

    devices = jax.devices()[:n_cores]
    mesh = Mesh(np.asarray(devices), ("core",))
    in_specs = (PartitionSpec("core"),) * (n_params + n_outs)
    out_specs = (PartitionSpec("core"),) * n_outs
    sharded = jax.jit(
        shard_map(_body, mesh=mesh, in_specs=in_specs, out_specs=out_specs,
                  check_rep=False),
        keep_unused=True,
    )
    concat_in = [
        np.concatenate([np.asarray(in_maps[c][nm]) for c in range(n_cores)], axis=0)
        for nm in in_names
    ]
    concat_zeros = [np.zeros((n_cores * z.shape[0], *z.shape[1:]), z.dtype)
                    for z in zero_outs]
    args = concat_in + concat_zeros
    out_arrs = sharded(*args)
    jax.block_until_ready(out_arrs)

    if time_reps > 0:
        sh = jax.sharding.NamedSharding(mesh, PartitionSpec("core"))
        dev_args = [jax.device_put(a, sh) for a in args]
        jax.block_until_ready(dev_args)
        jax.block_until_ready(sharded(*dev_args))  # warm
        t0 = _time.perf_counter()
        o = None
        for _ in range(time_reps):
            o = sharded(*dev_args)
        jax.block_until_ready(o)
        dt = (_time.perf_counter() - t0) / time_reps
        ns = int(dt * 1e9)
        _NC_CACHE["exec_time_ns"] = ns
        print(f"HW exec time: {ns} ns   (warm wall avg over {time_reps} reps)")

    results = [
        {name: np.asarray(out_arrs[i]).reshape(n_cores, *out_avals[i].shape)[c]
         for i, name in enumerate(out_names)}
        for c in range(n_cores)
    ]
    return _Res(results)


def kernel(x, W0, b0, W1, b1, W2, b2, codebooks):
    if "nc" not in _NC_CACHE:
        _NC_CACHE["nc"] = build_kernel()
    nc = _NC_CACHE["nc"]

    in_maps = _host_prep(x, W0, b0, W1, b1, W2, b2, codebooks)
    reps = int(os.environ.get("KERNEL_TIME_REPS", "0"))
    res = _run_pjrt(nc, in_maps, N_CORES, time_reps=reps)

    x_q = np.empty((N_TOTAL, E), np.float32)
    codes = np.empty((L, N_TOTAL), np.int32)
    total_znorm = 0.0
    msums = np.zeros(L, dtype=np.float64)
    for c in range(N_CORES):
        out = res.results[c]
        x_q[c * N_PER_CORE:(c + 1) * N_PER_CORE] = out["xqT"].T
        codes[:, c * N_PER_CORE:(c + 1) * N_PER_CORE] = \
            np.rint(out["idx"].astype(np.float32)).astype(np.int32)
        total_znorm += float((out["zT"].astype(np.float64) ** 2).sum())
        mv = out["mv"].astype(np.float64).reshape(128, N_TILES, L, NC)
        msums += mv.sum(axis=(0, 1, 3))

    # loss telescope: sum_n |r_{l+1}|^2 = sum|z|^2 - sum_{j<=l} msum_j
    denom = float(N_TOTAL * E)
    losses = []
    run = total_znorm
    for l in range(L):
        run -= msums[l]
        losses.append((1.0 + BETA) * run / denom)
    rq_loss = np.float32(np.mean(losses))

    return x_q, rq_loss, codes


if __name__ == "__main__":
    import reference
    inputs = reference.setup_inputs()
    inputs = {k: np.asarray(v) for k, v in inputs.items()}
    out = kernel(**inputs)
    print("x_q", out[0].shape, "loss", out[1], "codes", out[2].shape)


# revision 28
# speedup vs baseline: 1.2741x; 1.2741x over previous
"""CRQVAE (encoder MLP + 4-level residual VQ) on 8 TRN2 NeuronCores.

Data-parallel: batch N=131072 sharded 8 ways (16384 rows/core); encoder
weights + codebooks replicated. The scalar rq_loss is reduced on the host
from tiny per-core partials (the only cross-core communication).

Device design (per core, feature-major activations, 32 tiles x 512 cols,
software-pipelined so tile i+1's encoder overlaps tile i's RVQ):
  - All large matmuls run as 3-term float32r splits (a = ah + al with
    ah = round-to-11-explicit-mantissa-bits, the measured HW f32r width):
    ah.bh + ah.bl + al.bh at 1 cyc/row gives ~fp32 accuracy at 3/4 the
    fp32-matmul instruction count and 1/4 its per-row cost (fp32 matmul is
    4 cyc/row on TRN2). Weight/x splits are precomputed on the host;
    hidden-activation splits on device (ACT rounding copy + GpSimd sub).
  - RVQ per level: batch-major scores s = 2 r.c - |c|^2 (3-term f32r,
    residual stationary), DVE row-max m, one-hot = is_ge(s, m) on the SAME
    tensor (exact argmax one-hot by construction, matching jnp.argmin up
    to fp32-level rounding; exact-ties are vanishingly rare), PE-transpose
    of the bf16 one-hot to code-major, q/idx gather as a 3-term bf16 split
    of [cb | 0 | iota] (exact, since one-hot is {0,1} and bf16^3 recovers
    all 24 fp32 mantissa bits), residual update r -= q on DVE in fp32.
  - Outputs: x_qT = zT - r4T, zT (both feature-major), idx rows (bf16,
    exact integers), m row-maxima (for the host-side loss telescope
    sum_n |r_{l+1}|^2 = sum_n |z|^2 - sum_{j<=l} sum_n m_j, since
    |r - q|^2 = |r|^2 - s_max for the argmax code).
"""
import os
import sys

sys.path.insert(0, "/opt/trn_rl_repo")

import numpy as np

import concourse.bass as bass
import concourse.tile as tile
from concourse import bacc, mybir
from concourse import bass_utils

F32 = mybir.dt.float32
F32R = mybir.dt.float32r
BF16 = mybir.dt.bfloat16
AF = mybir.ActivationFunctionType

N_CORES = 8
N_TOTAL = 131072
N_PER_CORE = N_TOTAL // N_CORES        # 16384
B = 512                                # batch columns per tile
N_TILES = N_PER_CORE // B              # 32
D0, D1, D2, E = 768, 512, 256, 64
K = 256                                # codebook size
L = 4                                  # rvq levels
BETA = 0.25

KC1, KC2, KC3 = D0 // 128, D1 // 128, D2 // 128   # 6, 4, 2
JC1, JC2 = D1 // 128, D2 // 128                   # 4, 2
NC = B // 128                                     # 4 n-chunks per tile
KCC = K // 128                                    # 2 code chunks


def build_kernel(n_tiles=N_TILES, n_levels=L):
    nc = bacc.Bacc("TRN2", target_bir_lowering=False, debug=False)
    n = n_tiles * B

    # ---------------- DRAM parameters ----------------
    xh_d = nc.dram_tensor("xh", [D0, n], F32R, kind="ExternalInput")
    xl_d = nc.dram_tensor("xl", [D0, n], F32R, kind="ExternalInput")
    w0h_d = nc.dram_tensor("w0h", [D0, D1], F32R, kind="ExternalInput")
    w0l_d = nc.dram_tensor("w0l", [D0, D1], F32R, kind="ExternalInput")
    w1h_d = nc.dram_tensor("w1h", [D1, D2], F32R, kind="ExternalInput")
    w1l_d = nc.dram_tensor("w1l", [D1, D2], F32R, kind="ExternalInput")
    w2h_d = nc.dram_tensor("w2h", [D2, E + 1], F32R, kind="ExternalInput")
    w2l_d = nc.dram_tensor("w2l", [D2, E + 1], F32R, kind="ExternalInput")
    b0_d = nc.dram_tensor("b0", [128, JC1], F32, kind="ExternalInput")
    b1_d = nc.dram_tensor("b1", [128, JC2], F32, kind="ExternalInput")
    b2_d = nc.dram_tensor("b2", [E + 1, 1], F32, kind="ExternalInput")   # [b2;1]
    cbah_d = nc.dram_tensor("cbah", [L, E + 1, K], F32R, kind="ExternalInput")
    cbal_d = nc.dram_tensor("cbal", [L, E + 1, K], F32R, kind="ExternalInput")
    cbx_d = nc.dram_tensor("cbx", [3, L, K, 97], BF16, kind="ExternalInput")
    eyeb_d = nc.dram_tensor("eyeb", [128, 128], BF16, kind="ExternalInput")

    xqT_d = nc.dram_tensor("xqT", [E, n], F32, kind="ExternalOutput")
    zT_d = nc.dram_tensor("zT", [E, n], F32, kind="ExternalOutput")
    idx_d = nc.dram_tensor("idx", [L, n], BF16, kind="ExternalOutput")
    mv_d = nc.dram_tensor("mv", [128, n_tiles * L * NC], F32, kind="ExternalOutput")

    with tile.TileContext(nc) as tc:
        with (
            tc.tile_pool(name="const", bufs=1) as cp,
            tc.tile_pool(name="xin", bufs=2) as xp,
            tc.tile_pool(name="act", bufs=2) as ap,
            tc.tile_pool(name="rres", bufs=5) as rp,
            tc.tile_pool(name="oh", bufs=2) as ohp,
            tc.tile_pool(name="stage", bufs=2) as stp,
            tc.tile_pool(name="pmlp", bufs=2, space="PSUM") as pmlp,
            tc.tile_pool(name="pbs", bufs=1, space="PSUM") as pbs,
            tc.tile_pool(name="pfs", bufs=2, space="PSUM") as pfs,
            tc.tile_pool(name="pq", bufs=2, space="PSUM") as pq,
        ):
            # ---- constants (loaded once) ----
            w0h = cp.tile([128, KC1, D1], F32R, tag="w0h")
            w0l = cp.tile([128, KC1, D1], F32R, tag="w0l")
            w1h = cp.tile([128, KC2, D2], F32R, tag="w1h")
            w1l = cp.tile([128, KC2, D2], F32R, tag="w1l")
            w2h = cp.tile([128, KC3, E + 1], F32R, tag="w2h")
            w2l = cp.tile([128, KC3, E + 1], F32R, tag="w2l")
            b0c = cp.tile([128, JC1], F32, tag="b0")
            b1c = cp.tile([128, JC2], F32, tag="b1")
            b2c = cp.tile([E + 1, 1], F32, tag="b2")
            eyeb = cp.tile([128, 128], BF16, tag="eyeb")
            cbah = [cp.tile([E + 1, K], F32R, tag=f"cbah{l}", name=f"cbah{l}")
                    for l in range(L)]
            cbal = [cp.tile([E + 1, K], F32R, tag=f"cbal{l}", name=f"cbal{l}")
                    for l in range(L)]
            cbx = [cp.tile([128, KCC, 3, 97], BF16, tag=f"cbx{l}", name=f"cbx{l}")
                   for l in range(L)]

            for c in range(KC1):
                nc.sync.dma_start(w0h[:, c, :], w0h_d[c * 128:(c + 1) * 128, :])
                nc.sync.dma_start(w0l[:, c, :], w0l_d[c * 128:(c + 1) * 128, :])
            for c in range(KC2):
                nc.sync.dma_start(w1h[:, c, :], w1h_d[c * 128:(c + 1) * 128, :])
                nc.sync.dma_start(w1l[:, c, :], w1l_d[c * 128:(c + 1) * 128, :])
            for c in range(KC3):
                nc.sync.dma_start(w2h[:, c, :], w2h_d[c * 128:(c + 1) * 128, :])
                nc.sync.dma_start(w2l[:, c, :], w2l_d[c * 128:(c + 1) * 128, :])
            nc.sync.dma_start(b0c[:], b0_d[:])
            nc.sync.dma_start(b1c[:], b1_d[:])
            nc.sync.dma_start(b2c[:], b2_d[:])
            nc.sync.dma_start(eyeb[:], eyeb_d[:])
            for l in range(L):
                nc.sync.dma_start(cbah[l][:], cbah_d[l])
                nc.sync.dma_start(cbal[l][:], cbal_d[l])
                for t in range(3):
                    for kc in range(KCC):
                        nc.sync.dma_start(cbx[l][:, kc, t, :],
                                          cbx_d[t, l, kc * 128:(kc + 1) * 128, :])

            def mm3(pm, wh, wl, rh, rl, nk):
                """3-term f32r accumulation over nk contraction chunks.
                All rh-terms first so the matmuls can start before rl
                (the device-side lo-split) is ready."""
                first = True
                for c in range(nk):
                    for wt in (wh, wl):
                        nc.tensor.matmul(pm, wt[0][:, c, wt[1]], rh[:, c, :],
                                         start=first, stop=False)
                        first = False
                for c in range(nk):
                    nc.tensor.matmul(pm, wh[0][:, c, wh[1]], rl[:, c, :],
                                     start=False, stop=(c == nk - 1))

            def encoder(i):
                """Tile i: DMA x, MLP -> r0 = [z + b2; 1] in SBUF."""
                sl = slice(i * B, (i + 1) * B)
                xth = xp.tile([128, KC1, B], F32R, tag="xh")
                xtl = xp.tile([128, KC1, B], F32R, tag="xl")
                nc.sync.dma_start(
                    xth[:], xh_d[:, sl].rearrange("(c p) n -> p c n", c=KC1))
                nc.sync.dma_start(
                    xtl[:], xl_d[:, sl].rearrange("(c p) n -> p c n", c=KC1))

                h1 = ap.tile([128, JC1, B], F32, tag="h1")
                h1h = ap.tile([128, JC1, B], F32R, tag="h1h")
                h1l = ap.tile([128, JC1, B], F32R, tag="h1l")
                for j in range(JC1):
                    pm = pmlp.tile([128, B], F32, tag="pm")
                    jsl = slice(j * 128, (j + 1) * 128)
                    mm3(pm[:], (w0h, jsl), (w0l, jsl), xth, xtl, KC1)
                    nc.scalar.activation(h1[:, j, :], pm[:], AF.Relu,
                                         bias=b0c[:, j:j + 1], scale=1.0)
                    nc.scalar.copy(h1h[:, j, :], h1[:, j, :])
                    nc.gpsimd.tensor_tensor(h1l[:, j, :], h1[:, j, :],
                                            h1h[:, j, :],
                                            op=mybir.AluOpType.subtract)

                h2 = ap.tile([128, JC2, B], F32, tag="h2", bufs=1)
                h2h = ap.tile([128, JC2, B], F32R, tag="h2h", bufs=2)
                h2l = ap.tile([128, JC2, B], F32R, tag="h2l", bufs=2)
                for j in range(JC2):
                    pm = pmlp.tile([128, B], F32, tag="pm")
                    jsl = slice(j * 128, (j + 1) * 128)
                    mm3(pm[:], (w1h, jsl), (w1l, jsl), h1h, h1l, KC2)
                    nc.scalar.activation(h2[:, j, :], pm[:], AF.Relu,
                                         bias=b1c[:, j:j + 1], scale=1.0)
                    nc.scalar.copy(h2h[:, j, :], h2[:, j, :])
                    nc.gpsimd.tensor_tensor(h2l[:, j, :], h2[:, j, :],
                                            h2h[:, j, :],
                                            op=mybir.AluOpType.subtract)

                pzt = pmlp.tile([E + 1, B], F32, tag="pm")
                asl = slice(0, E + 1)
                mm3(pzt[:], (w2h, asl), (w2l, asl), h2h, h2l, KC3)
                r0 = rp.tile([E + 1, B], F32, tag="r")
                nc.scalar.activation(r0[:], pzt[:], AF.Identity,
                                     bias=b2c[:], scale=1.0)
                nc.sync.dma_start(zT_d[:, sl], r0[0:E, :])
                return r0

            def rvq(i, r0):
                sl = slice(i * B, (i + 1) * B)
                mst = stp.tile([128, L * NC], F32, tag="mst", bufs=2)
                r_cur = r0
                for l in range(n_levels):
                    # split r into f32r hi/lo
                    rh = rp.tile([E + 1, B], F32R, tag="rh")
                    nc.scalar.copy(rh[:], r_cur[:])
                    rl = rp.tile([E + 1, B], F32R, tag="rl")
                    nc.gpsimd.tensor_tensor(rl[:], r_cur[:], rh[:],
                                            op=mybir.AluOpType.subtract)

                    # ---- batch-major scores s = 2 r.c - |c|^2 -> row max ----
                    # rh-terms for all chunks first; rl-terms last (rl is
                    # produced later by the GpSimd lo-split).
                    pb = pbs.tile([128, NC, K], F32, tag="pb")
                    for c in range(NC):
                        csl = slice(c * 128, (c + 1) * 128)
                        nc.tensor.matmul(pb[:, c, :], rh[:, csl], cbah[l][:],
                                         start=True, stop=False)
                        nc.tensor.matmul(pb[:, c, :], rh[:, csl], cbal[l][:],
                                         start=False, stop=False)
                    for c in range(NC):
                        csl = slice(c * 128, (c + 1) * 128)
                        nc.tensor.matmul(pb[:, c, :], rl[:, csl], cbah[l][:],
                                         start=False, stop=True)
                    nc.vector.tensor_reduce(mst[:, l * NC:(l + 1) * NC], pb[:],
                                            axis=mybir.AxisListType.X,
                                            op=mybir.AluOpType.max)

                    # ---- exact one-hot: (s >= rowmax), bf16 ----
                    oh = ohp.tile([128, NC, K], BF16, tag="oh")
                    for c in range(NC):
                        nc.vector.tensor_scalar(
                            oh[:, c, :], pb[:, c, :],
                            scalar1=mst[:, l * NC + c:l * NC + c + 1],
                            scalar2=None, op0=mybir.AluOpType.is_ge)

                    # ---- transpose one-hot to code-major (PE, bf16) ----
                    pt = pfs.tile([128, KCC, B], BF16, tag="pt")
                    for c in range(NC):
                        for kc in range(KCC):
                            nc.tensor.transpose(
                                pt[:, kc, c * 128:(c + 1) * 128],
                                oh[:, c, kc * 128:(kc + 1) * 128], eyeb[:])
                    ohT = ohp.tile([128, KCC, B], BF16, tag="ohT")
                    nc.scalar.copy(ohT[:], pt[:])

                    # ---- q / idx gather: 3-term bf16 split of cb (exact) ----
                    pqt = pq.tile([97, B], F32, tag="pq")
                    for t in range(3):
                        for kc in range(KCC):
                            nc.tensor.matmul(pqt[:], cbx[l][:, kc, t, :],
                                             ohT[:, kc, :],
                                             start=(t == 0 and kc == 0),
                                             stop=(t == 2 and kc == KCC - 1))
                    ixl = stp.tile([1, B], BF16, tag=f"ist{l}", name=f"ist{l}", bufs=1)
                    nc.scalar.copy(ixl[:], pqt[96:97, :])
                    nc.gpsimd.dma_start(idx_d[l:l + 1, sl], ixl[:])

                    # ---- residual update (rows 0..64; row64: 1-0=1) ----
                    r_nxt = rp.tile([E + 1, B], F32, tag="r")
                    nc.vector.tensor_tensor(r_nxt[:], r_cur[:], pqt[0:E + 1, :],
                                            op=mybir.AluOpType.subtract)
                    r_cur = r_nxt

                # ---- x_q = z - r4 ----
                xq = stp.tile([E, B], F32, tag="xq", bufs=1)
                nc.vector.tensor_tensor(xq[:], r0[0:E, :], r_cur[0:E, :],
                                        op=mybir.AluOpType.subtract)
                nc.gpsimd.dma_start(xqT_d[:, sl], xq[:])
                nc.gpsimd.dma_start(mv_d[:, i * L * NC:(i + 1) * L * NC], mst[:])

            # software pipeline: encoder runs one tile ahead of rvq
            r_prev = encoder(0)
            for i in range(n_tiles):
                r_next = encoder(i + 1) if i + 1 < n_tiles else None
                rvq(i, r_prev)
                r_prev = r_next

    nc.compile()
    return nc


def _r11(x):
    """Round fp32 to 11 explicit mantissa bits (matches HW f32r width)."""
    i = x.view(np.int32).astype(np.int64)
    i = (i + (1 << 11)) & ~((1 << 12) - 1)
    return np.asarray(i, np.int64).astype(np.int32).view(np.float32)


def _split(x):
    h = _r11(np.ascontiguousarray(x, np.float32))
    l = (x - h).astype(np.float32)
    return h, np.ascontiguousarray(l)


def _host_prep(x, W0, b0, W1, b1, W2, b2, codebooks):
    """Build per-core input maps."""
    import ml_dtypes
    cb = np.asarray(codebooks, np.float32)                    # [4,256,64]
    cn32 = ((cb ** 2).sum(-1)).astype(np.float32)             # fp32 |c|^2

    cba = np.zeros((L, E + 1, K), np.float32)
    cba[:, :E, :] = np.transpose(cb, (0, 2, 1)) * 2.0
    cba[:, E, :] = -cn32
    cbah, cbal = _split(cba)

    cbf = np.zeros((L, K, 97), np.float32)
    cbf[:, :, :E] = cb
    cbf[:, :, 96] = np.arange(K, dtype=np.float32)[None, :]
    h = cbf.astype(ml_dtypes.bfloat16)
    r1 = cbf - h.astype(np.float32)
    m_ = r1.astype(ml_dtypes.bfloat16)
    r2 = r1 - m_.astype(np.float32)
    lo = r2.astype(ml_dtypes.bfloat16)
    cbx = np.stack([h, m_, lo])                               # [3,L,K,97] bf16
    eyeb = np.eye(128, dtype=ml_dtypes.bfloat16)

    w2a = np.zeros((D2, E + 1), np.float32)
    w2a[:, :E] = np.asarray(W2, np.float32)
    b2a = np.zeros((E + 1, 1), np.float32)
    b2a[:E, 0] = np.asarray(b2, np.float32)
    b2a[E, 0] = 1.0

    w0h, w0l = _split(np.asarray(W0, np.float32))
    w1h, w1l = _split(np.asarray(W1, np.float32))
    w2h, w2l = _split(w2a)

    common = {
        "w0h": w0h, "w0l": w0l, "w1h": w1h, "w1l": w1l,
        "w2h": w2h, "w2l": w2l,
        "b0": np.ascontiguousarray(np.asarray(b0, np.float32).reshape(JC1, 128).T),
        "b1": np.ascontiguousarray(np.asarray(b1, np.float32).reshape(JC2, 128).T),
        "b2": b2a,
        "cbah": cbah, "cbal": cbal, "cbx": cbx, "eyeb": eyeb,
    }

    x = np.asarray(x, np.float32)
    in_maps = []
    for c in range(N_CORES):
        xs = x[c * N_PER_CORE:(c + 1) * N_PER_CORE]
        xT = np.ascontiguousarray(xs.T)
        xh, xl = _split(xT)
        m = dict(common)
        m["xh"] = xh
        m["xl"] = xl
        in_maps.append(m)
    return in_maps


_NC_CACHE = {}


class _Res:
    def __init__(self, results):
        self.results = results


def _run_pjrt(nc, in_maps, n_cores, time_reps=0):
    """Execute the compiled Bass graph on n_cores via PJRT (axon), optionally
    timing warm repeat executions with device-resident inputs."""
    import time as _time
    import jax
    from jax.sharding import Mesh, PartitionSpec
    from jax.experimental.shard_map import shard_map
    from concourse import bass2jax, mybir as _mb
    from concourse.bass2jax import _bass_exec_p, install_neuronx_cc_hook

    install_neuronx_cc_hook()
    partition_name = nc.partition_id_tensor.name if nc.partition_id_tensor else None

    in_names, out_names, out_avals, zero_outs = [], [], [], []
    for alloc in nc.m.functions[0].allocations:
        if not isinstance(alloc, _mb.MemoryLocationSet):
            continue
        name = alloc.memorylocations[0].name
        if alloc.kind == "ExternalInput":
            if name != partition_name:
                in_names.append(name)
        elif alloc.kind == "ExternalOutput":
            shape = tuple(alloc.tensor_shape)
            dtype = _mb.dt.np(alloc.dtype)
            out_names.append(name)
            out_avals.append(jax.core.ShapedArray(shape, dtype))
            zero_outs.append(np.zeros(shape, dtype))
    n_params = len(in_names)
    n_outs = len(out_avals)
    all_in_names = list(in_names) + list(out_names)
    if partition_name is not None:
        all_in_names.append(partition_name)

    def _body(*args):
        operands = list(args)
        if partition_name is not None:
            operands.append(bass2jax.partition_id_tensor())
        outs = _bass_exec_p.bind(
            *operands,
            out_avals=tuple(out_avals),
            in_names=tuple(all_in_names),
            out_names=tuple(out_names),
            lowering_input_output_aliases=(),
            sim_require_finite=True,
            sim_require_nnan=True,
            nc=nc,
        )
        return tuple(outs)

    devices = jax.devices()[:n_cores]
    mesh = Mesh(np.asarray(devices), ("core",))
    in_specs = (PartitionSpec("core"),) * (n_params + n_outs)
    out_specs = (PartitionSpec("core"),) * n_outs
    sharded = jax.jit(
        shard_map(_body, mesh=mesh, in_specs=in_specs, out_specs=out_specs,
                  check_rep=False),
        keep_unused=True,
    )
    concat_in = [
        np.concatenate([np.asarray(in_maps[c][nm]) for c in range(n_cores)], axis=0)
        for nm in in_names
    ]
    concat_zeros = [np.zeros((n_cores * z.shape[0], *z.shape[1:]), z.dtype)
                    for z in zero_outs]
    args = concat_in + concat_zeros
    out_arrs = sharded(*args)
    jax.block_until_ready(out_arrs)

    if time_reps > 0:
        sh = jax.sharding.NamedSharding(mesh, PartitionSpec("core"))
        dev_args = [jax.device_put(a, sh) for a in args]
        jax.block_until_ready(dev_args)
        jax.block_until_ready(sharded(*dev_args))  # warm
        t0 = _time.perf_counter()
        o = None
        for _ in range(time_reps):
            o = sharded(*dev_args)
        jax.block_until_ready(o)
        dt = (_time.perf_counter() - t0) / time_reps
        ns = int(dt * 1e9)
        _NC_CACHE["exec_time_ns"] = ns
        print(f"HW exec time: {ns} ns   (warm wall avg over {time_reps} reps)")

    results = [
        {name: np.asarray(out_arrs[i]).reshape(n_cores, *out_avals[i].shape)[c]
         for i, name in enumerate(out_names)}
        for c in range(n_cores)
    ]
    return _Res(results)


def kernel(x, W0, b0, W1, b1, W2, b2, codebooks):
    if "nc" not in _NC_CACHE:
        _NC_CACHE["nc"] = build_kernel()
    nc = _NC_CACHE["nc"]

    in_maps = _host_prep(x, W0, b0, W1, b1, W2, b2, codebooks)
    reps = int(os.environ.get("KERNEL_TIME_REPS", "0"))
    res = _run_pjrt(nc, in_maps, N_CORES, time_reps=reps)

    x_q = np.empty((N_TOTAL, E), np.float32)
    codes = np.empty((L, N_TOTAL), np.int32)
    total_znorm = 0.0
    msums = np.zeros(L, dtype=np.float64)
    for c in range(N_CORES):
        out = res.results[c]
        x_q[c * N_PER_CORE:(c + 1) * N_PER_CORE] = out["xqT"].T
        codes[:, c * N_PER_CORE:(c + 1) * N_PER_CORE] = \
            np.rint(out["idx"].astype(np.float32)).astype(np.int32)
        total_znorm += float((out["zT"].astype(np.float64) ** 2).sum())
        mv = out["mv"].astype(np.float64).reshape(128, N_TILES, L, NC)
        msums += mv.sum(axis=(0, 1, 3))

    # loss telescope: sum_n |r_{l+1}|^2 = sum|z|^2 - sum_{j<=l} msum_j
    denom = float(N_TOTAL * E)
    losses = []
    run = total_znorm
    for l in range(L):
        run -= msums[l]
        losses.append((1.0 + BETA) * run / denom)
    rq_loss = np.float32(np.mean(losses))

    return x_q, rq_loss, codes


if __name__ == "__main__":
    import reference
    inputs = reference.setup_inputs()
    inputs = {k: np.asarray(v) for k, v in inputs.items()}
    out = kernel(**inputs)
    print("x_q", out[0].shape, "loss", out[1], "codes", out[2].shape)
